# revision 18
# baseline (speedup 1.0000x reference)
"""CTC loss kernel for Trainium2 (8 NeuronCores, data-parallel over batch).

Problem: B=1024, T=384, C=96, L=48 (no -1 padding in labels by construction).
reference: mean_b of CTC forward negative-log-likelihood of predictions[b]
given labels[b], blank = C-1.

Strategy per core (128 examples = 128 SBUF partitions):
  Phase A (x16 batches of 8 examples spread over 16 t-slices each):
    - DMA predictions batch (spread layout), exp on ACT,
    - softmax denominators Z[b,t] via DVE reduce, log Z accumulated on ACT,
    - emission gather u_ext[b,t,s] = exp(pred)[b,t,ext[b,s]] via GPSIMD
      ap_gather (indices shared within each 16-partition group = 1 example),
    - reshuffle DMA (fp32->bf16 cast) into resident u_ext [128, T, S].
  Phase B: prob-domain CTC forward scan over t (alpha in fp32, renormalized
    every 8 steps by 1/max with log accumulation).
  Phase C: loss[b] = sum_t log Z[b,t] - log(alpha_T[S-1]+alpha_T[S-2]) - logacc[b].
Host combines the 8x128 per-example losses into the scalar mean.
"""

import os
import numpy as np

import concourse.bass as bass
import concourse.tile as tile
import concourse.mybir as mybir
from concourse.bass_utils import run_bass_kernel_spmd

# ---------------------------------------------------------------- constants
B, T, C, L = 1024, 384, 96, 48
S = 2 * L + 1            # 97
BLANK = C - 1            # 95
N_CORES = 8
PB = B // N_CORES        # 128 examples per core
NB = 16                  # gather batches per core (8 examples each)
NG = 8                   # gpsimd groups (16 partitions each)
TSL = 16                 # t-slices per example (one per partition in group)
TL = T // TSL            # 24 timesteps per slice
NE = TL * C              # 2304 elements per partition row (spread layout)
NI = TL * S              # 2328 gathered values per t-slice
NIP = 2336               # padded to a multiple of 16
RENORM_EVERY = 8

F32 = mybir.dt.float32
BF16 = mybir.dt.bfloat16
I16 = mybir.dt.uint16

_CACHE = {}


def _split_waits(nc, limit=1):
    """This walrus build rejects instructions with >1 sync-wait command.
    Move excess waits onto same-engine NOPs inserted just before."""
    n = 0
    for fn in nc.m.functions:
        for bb in fn.blocks:
            new_instrs = []
            for inst in bb.instructions:
                si = inst.sync_info
                if si is not None and si.on_wait and len(si.on_wait) > limit:
                    waits = list(si.on_wait)
                    chunks = [waits[i:i + limit] for i in range(0, len(waits), limit)]
                    for chunk in chunks[:-1]:
                        nop = mybir.InstNoOp(name=f"{inst.name}-ws{n}", ins=[], outs=[])
                        nop.engine = inst.engine
                        nop.sync_info = mybir.SyncInfo(on_wait=list(chunk), on_update=[])
                        nc.register_instruction(nop)
                        new_instrs.append(nop)
                        n += 1
                    si.on_wait = list(chunks[-1])
                new_instrs.append(inst)
            bb.instructions = new_instrs
    return n


def build_program():
    nc = bass.Bass("TRN2", target_bir_lowering=False, debug=False,
                   num_devices=N_CORES)
    pred_ap = nc.dram_tensor("pred", [PB, NB, NE], F32, kind="ExternalInput").ap()
    gidx_ap = nc.dram_tensor("gidx", [PB, NB, NIP // 16], I16, kind="ExternalInput").ap()
    mask_ap = nc.dram_tensor("mask", [PB, S], F32, kind="ExternalInput").ap()
    oneb_ap = nc.dram_tensor("oneblk", [PB, NG], F32, kind="ExternalInput").ap()
    loss_ap = nc.dram_tensor("loss", [PB, 1], F32, kind="ExternalOutput").ap()

    with tile.TileContext(nc) as tc:
        _build_body(tc, pred_ap, gidx_ap, mask_ap, oneb_ap, loss_ap)
    _split_waits(nc)
    return nc


def _build_body(tc, pred_ap, gidx_ap, mask_ap, oneb_ap, loss_ap):
    nc = tc.nc
    from contextlib import ExitStack
    with ExitStack() as ctx:
        singles = ctx.enter_context(tc.tile_pool(name="singles", bufs=1))
        ubuf = ctx.enter_context(tc.tile_pool(name="ubuf", bufs=1))
        batch_io = ctx.enter_context(tc.tile_pool(name="batch_io", bufs=2))
        small = ctx.enter_context(tc.tile_pool(name="small", bufs=2))
        psum = ctx.enter_context(tc.tile_pool(name="psum", bufs=1, space="PSUM"))

        # ---------------- resident tensors
        u_ext = ubuf.tile([PB, T, S], BF16)          # 73KB/partition
        gidx_sb = singles.tile([PB, NB, NIP // 16], I16)
        nc.sync.dma_start(out=gidx_sb, in_=gidx_ap)
        mask_sb = singles.tile([PB, S], F32)
        nc.sync.dma_start(out=mask_sb, in_=mask_ap)
        lzp = singles.tile([PB, NB], F32)            # per-(partition,batch) sum log Z
        ones_blk = singles.tile([PB, NG], F32)
        nc.sync.dma_start(out=ones_blk, in_=oneb_ap)

        # ---------------- phase A: exp, Z, gather, reshuffle
        for j in range(NB):
            psb = batch_io.tile([PB, NE], F32, tag="psb")
            nc.sync.dma_start(out=psb, in_=pred_ap[:, j, :])
            ue = batch_io.tile([PB, NE], F32, tag="ue")
            nc.scalar.activation(ue, psb, mybir.ActivationFunctionType.Exp)
            zt = small.tile([PB, TL], F32, tag="zt")
            nc.vector.tensor_reduce(
                out=zt, in_=ue.rearrange("p (t c) -> p t c", c=C),
                axis=mybir.AxisListType.X, op=mybir.AluOpType.add)
            lzs = small.tile([PB, TL], F32, tag="lzs")
            nc.scalar.activation(lzs, zt, mybir.ActivationFunctionType.Ln,
                                 accum_out=lzp[:, j:j + 1])
            uex = batch_io.tile([PB, NIP], F32, tag="uex")
            # IndirectCopy dst elem count is capped at 1024 on this compiler
            for st in range(0, NIP, 1024):
                cnt = min(1024, NIP - st)
                nc.gpsimd.indirect_copy(
                    uex[:, st:st + cnt], ue[:, :],
                    gidx_sb[:, j, st // 16:(st + cnt) // 16],
                    i_know_ap_gather_is_preferred=True)
            # cast to bf16 on DVE, then reshuffle:
            # 16 t-slices x 8 examples -> 8 example rows
            uexb = batch_io.tile([PB, NIP], BF16, tag="uexb")
            nc.vector.tensor_copy(uexb[:, 0:NI], uex[:, 0:NI])
            dst = u_ext[8 * j:8 * (j + 1)].rearrange(
                "p (q tl) s -> p q (tl s)", q=TSL)
            nc.sync.dma_start(out=dst, in_=uexb[:, 0:NI])

        # per-example sum over the 16 partition-slices of log-Z partials
        plz = psum.tile([NB, NG], F32)
        nc.tensor.matmul(plz, lzp, ones_blk, start=True, stop=True)
        slz_sb = small.tile([NB, NG], F32, tag="slz")
        nc.vector.tensor_copy(slz_sb, plz)
        sumlz = singles.tile([PB, 1], F32)
        nc.sync.dma_start(out=sumlz, in_=slz_sb)

        # ---------------- phase B: forward scan in prob domain
        apool = ctx.enter_context(tc.tile_pool(name="apool", bufs=2))
        spool = ctx.enter_context(tc.tile_pool(name="spool", bufs=2))
        rpool = ctx.enter_context(tc.tile_pool(name="rpool", bufs=2))

        logacc = singles.tile([PB, 1], F32)
        nc.vector.memset(logacc, 0.0)

        a_tiles = [apool.tile([PB, S + 2], F32, tag="alpha", name=f"alpha{i}")
                   for i in range(2)]
        for a in a_tiles:
            nc.vector.memset(a, 0.0)
        A = a_tiles[0]
        # init: alpha_0[s] = u_ext[0, s] for s in {0, 1}
        nc.vector.tensor_copy(A[:, 2:4], u_ext[:, 0, 0:2])

        for t in range(1, T):
            An = a_tiles[t % 2]
            t1 = spool.tile([PB, S], F32, tag="t1")
            nc.vector.tensor_add(t1, A[:, 2:S + 2], A[:, 1:S + 1])
            t2 = spool.tile([PB, S], F32, tag="t2")
            nc.vector.tensor_mul(t2, A[:, 0:S], mask_sb)
            nc.vector.tensor_add(t1, t1, t2)
            nc.vector.tensor_mul(An[:, 2:S + 2], t1, u_ext[:, t, :])
            A = An
            if t % RENORM_EVERY == 0:
                mx = rpool.tile([PB, 1], F32, tag="mx")
                nc.vector.tensor_reduce(out=mx, in_=A[:, 2:S + 2],
                                        axis=mybir.AxisListType.X,
                                        op=mybir.AluOpType.max)
                rc = rpool.tile([PB, 1], F32, tag="rc")
                nc.vector.reciprocal(rc, mx)
                nc.vector.tensor_scalar_mul(A[:, 2:S + 2], A[:, 2:S + 2], rc)
                lm = rpool.tile([PB, 1], F32, tag="lm")
                nc.scalar.activation(lm, mx, mybir.ActivationFunctionType.Ln)
                nc.vector.tensor_add(logacc, logacc, lm)

        # ---------------- phase C: finalize per-example loss
        esum = small.tile([PB, 1], F32, tag="esum")
        nc.vector.tensor_add(esum, A[:, S:S + 1], A[:, S + 1:S + 2])
        lg = small.tile([PB, 1], F32, tag="lg")
        nc.scalar.activation(lg, esum, mybir.ActivationFunctionType.Ln)
        tot = small.tile([PB, 1], F32, tag="tot")
        nc.vector.tensor_add(tot, lg, logacc)
        lossv = small.tile([PB, 1], F32, tag="lossv")
        nc.vector.tensor_sub(lossv, sumlz, tot)
        nc.sync.dma_start(out=loss_ap, in_=lossv)


# ------------------------------------------------------------------ host side

def host_inputs(labels, predictions):
    """Per-core input arrays (spread layout + gather indices + allow2 mask)."""
    labels = np.asarray(labels)
    predictions = np.asarray(predictions, dtype=np.float32)
    ins = []
    i_idx = np.arange(NIP)
    tl_i = np.minimum(i_idx // S, TL - 1)
    s_i = i_idx % S
    valid_i = i_idx < NI
    for c in range(N_CORES):
        lab = labels[c * PB:(c + 1) * PB].astype(np.int64)
        P = predictions[c * PB:(c + 1) * PB]
        # ext labels [PB, S]
        ext = np.full((PB, S), BLANK, dtype=np.int64)
        ext[:, 1::2] = lab
        # allow2 mask (multiplicative)
        same = np.zeros((PB, S), dtype=bool)
        same[:, 2:] = ext[:, 2:] == ext[:, :-2]
        is_lab = (np.arange(S) % 2 == 1)[None, :]
        mask = (is_lab & ~same).astype(np.float32)
        # spread predictions [128, NB, NE]; local example l = 8*j + g sits in
        # partition group g (16 partitions, one per t-slice) of batch j.
        P6 = P.reshape(NB, NG, TSL, TL, C)           # (j, g, p16, tl, c)
        pred_spread = np.ascontiguousarray(
            P6.transpose(1, 2, 0, 3, 4).reshape(PB, NB, NE))
        # gather indices
        vals = np.where(valid_i[None, :],
                        tl_i[None, :] * C + ext[:, s_i],
                        0).astype(np.uint16)          # [PB(b_local), NIP]
        gidx = np.zeros((PB, NB, NIP // 16), dtype=np.uint16)
        for g in range(NG):
            for j in range(NB):
                b_local = 8 * j + g
                gidx[16 * g:16 * (g + 1), j, :] = (
                    vals[b_local].reshape(NIP // 16, 16).T)
        oneblk = np.zeros((PB, NG), dtype=np.float32)
        for g in range(NG):
            oneblk[16 * g:16 * (g + 1), g] = 1.0
        ins.append({"pred": pred_spread, "gidx": gidx,
                    "mask": np.ascontiguousarray(mask), "oneblk": oneblk})
    return ins


def kernel(labels, predictions):
    if "nc" not in _CACHE:
        _CACHE["nc"] = build_program()
    nc = _CACHE["nc"]
    in_maps = host_inputs(labels, predictions)
    res = run_bass_kernel_spmd(
        nc, in_maps, core_ids=list(range(N_CORES)),
        trace=bool(int(os.environ.get("CTC_TRACE", "0"))))
    _CACHE["last_result"] = res
    losses = np.concatenate(
        [res.results[c]["loss"].reshape(PB) for c in range(N_CORES)])
    _CACHE["last_losses"] = losses
    return np.asarray(np.mean(losses), dtype=np.float32)
